# revision 1
# baseline (speedup 1.0000x reference)
"""GAT layer kernel for 8 Trainium2 NeuronCores.

Strategy (row-sharded attention, per the sharding hint):
  - Core c owns query rows [c*1024, (c+1)*1024) of the 8000-node graph
    (1024 = 8*128; core 7's slice is host-padded with zero rows; the key
    axis is padded to 8064 = 63*128 with zero adjacency columns).
  - Shard layout choice: each core's adjacency slice is uploaded
    TRANSPOSED ([8064 keys, 1024 queries] int32) so the attention matrix
    is built directly with keys on partitions — the layout the TensorE
    contraction needs. The kernel still streams the same 32 MB of int32
    adjacency per core from HBM (the memory-bound term is unchanged);
    SWDGE casts it to {0,1} fp16 in flight.
  - H = X @ W_w^T is computed replicated on every core from a host-
    transposed bf16 X. W_b is folded into the final output (softmax rows
    sum to 1 => attn @ (H0 + 1 W_b^T) = attn@H0 + W_b); its contribution
    to the scores goes through host-folded constants c_src/c_dst.
  - Scores in transposed layout: e = s_src[i] (broadcast tile, built by
    a rank-1 PE matmul) + s_dst[j] (per-partition scalar) on VectorE;
    leakyrelu as (0.2*e) max e in one scalar_tensor_tensor; exp on
    ScalarE (no max-subtraction: |e| <~ 3 so exp is safe); mask applied
    after exp (mask*exp(lrelu(e)) is exactly the reference's
    mask->-inf->lrelu->softmax weights) as one GpSimd multiply.
  - attn_unnorm @ [H | 1] runs as fp16 matmuls accumulating into 8 PSUM
    banks (one per 126-row output block); column 256 gives the softmax
    denominator, applied as a reciprocal per-partition multiply (plus
    the W_b add) while draining PSUM.
"""
import sys

sys.path.insert(0, "/opt/trn_rl_repo")

import numpy as np
import ml_dtypes

N, F = 8000, 256
NP = 8064          # padded key count (63 * 128)
W = 1024           # query rows per core (8 * 128; last core mostly padding)
NJT = NP // 128    # 63 key tiles
MB = W // 8        # 128-row output blocks (full-width weights enable FWL)
NEG_SLOPE = 0.2

_RUNNER = None
_last_in_maps = None


def _build(repeat=1):
    import concourse.bass as bass
    import concourse.tile as tile
    from concourse import bacc, mybir

    f16 = mybir.dt.float16
    f32 = mybir.dt.float32
    bf16 = mybir.dt.bfloat16

    nc = bacc.Bacc()
    adjtd = nc.dram_tensor("adjt", (NP, W), mybir.dt.int32, kind="ExternalInput")
    xtd = nc.dram_tensor("xtb", (F, NP), bf16, kind="ExternalInput")
    xwtd = nc.dram_tensor("xwtb", (F, W), bf16, kind="ExternalInput")
    wgd = nc.dram_tensor("wg", (F, 258), bf16, kind="ExternalInput")
    wbd = nc.dram_tensor("wbt", (128, F), f16, kind="ExternalInput")
    cvd = nc.dram_tensor("cv", (128, 2), f32, kind="ExternalInput")
    outd = nc.dram_tensor("out", (W, F), f32, kind="ExternalOutput")

    with tile.TileContext(nc) as tc:
        with (
            tc.tile_pool(name="pp", bufs=1) as pp,
            tc.tile_pool(name="att", bufs=2) as ap_,
            tc.tile_pool(name="fin", bufs=2) as fin,
            tc.tile_pool(name="ps", bufs=8, space="PSUM") as psp,
        ):
            for _rep in range(repeat):
                # ---- phase 0a: parameters and transposed activations ----
                wg_sb = [pp.tile([128, 258], bf16, name=f"wg{k}", tag=f"wg{k}") for k in range(2)]
                xt = [pp.tile([128, NP], bf16, name=f"xt{k}", tag=f"xt{k}") for k in range(2)]
                xwt = [pp.tile([128, W], bf16, name=f"xwt{k}", tag=f"xwt{k}") for k in range(2)]
                for k in range(2):
                    nc.sync.dma_start(wg_sb[k][:], wgd[k * 128 : (k + 1) * 128, :])
                    nc.sync.dma_start(xt[k][:], xtd[k * 128 : (k + 1) * 128, :])
                    nc.sync.dma_start(xwt[k][:], xwtd[k * 128 : (k + 1) * 128, :])
                wb_sb = pp.tile([128, F], f16)
                nc.sync.dma_start(wb_sb[:], wbd[:])
                cv_sb = pp.tile([128, 2], f32)
                nc.sync.dma_start(cv_sb[:], cvd[:])

                # ---- phase 0b: s_src row (this core's queries) ----
                ssrc_row = pp.tile([1, W], f32)
                for ch in range(2):
                    ps = psp.tile([2, 512], f32, name="ps_s", tag="ps")
                    for k in range(2):
                        nc.tensor.matmul(
                            ps[:],
                            wg_sb[k][:, 256:258],
                            xwt[k][:, ch * 512 : (ch + 1) * 512],
                            start=(k == 0),
                            stop=(k == 1),
                        )
                    nc.vector.tensor_scalar_add(
                        ssrc_row[0:1, ch * 512 : (ch + 1) * 512], ps[0:1, :], cv_sb[0:1, 0:1]
                    )

                # ---- phase 0c: broadcast s_src across partitions via PE ----
                ones1 = pp.tile([1, 128], f32)
                nc.vector.memset(ones1[:], 1.0)
                sb1 = pp.tile([128, W], f16)
                for ch in range(2):
                    psb_t = psp.tile([128, 512], f32, name="ps_b", tag="ps")
                    nc.tensor.matmul(
                        psb_t[:], ones1[:], ssrc_row[0:1, ch * 512 : (ch + 1) * 512],
                        start=True, stop=True,
                    )
                    nc.vector.tensor_copy(sb1[:, ch * 512 : (ch + 1) * 512], psb_t[:])

                # ---- phase 0d: H' key tiles + s_dst columns ----
                hp = pp.tile([128, NJT * 257], f16)
                sdst = pp.tile([128, NJT], f32)
                for jt in range(NJT):
                    ph = psp.tile([128, 258], f32, name="ps_h", tag="ps")
                    for k in range(2):
                        nc.tensor.matmul(
                            ph[:],
                            xt[k][:, jt * 128 : (jt + 1) * 128],
                            wg_sb[k][:],
                            start=(k == 0),
                            stop=(k == 1),
                        )
                    if jt % 2 == 0:
                        nc.vector.tensor_copy(hp[:, jt * 257 : jt * 257 + 256], ph[:, 0:256])
                    else:
                        nc.scalar.copy(hp[:, jt * 257 : jt * 257 + 256], ph[:, 0:256])
                    nc.scalar.activation(
                        sdst[:, jt : jt + 1], ph[:, 257:258],
                        mybir.ActivationFunctionType.Identity, bias=cv_sb[:, 1:2],
                    )
                    nc.vector.memset(hp[:, jt * 257 + 256 : jt * 257 + 257], 1.0)

                # ---- phase 1: masked attention weights + matmul accumulate ----
                # 4 key tiles per round: one 2 MB cast-DMA, then wide
                # stt/exp/mask ops amortize per-instruction overheads.
                po = [psp.tile([MB, 257], f32, name=f"po{ib}", tag="ps") for ib in range(8)]
                GR = 4
                groups = [list(range(g, min(g + GR, NJT))) for g in range(0, NJT, GR)]
                for jts in groups:
                    na = len(jts)
                    cw = na * W
                    j0 = jts[0]
                    adjT = ap_.tile([128, GR * W], f16, name="adjT", tag="adjT", bufs=3)
                    nc.gpsimd.dma_start(
                        adjT[:].rearrange("p (a w) -> p a w", w=W)[:, 0:na, :],
                        adjtd[j0 * 128 : (j0 + na) * 128, :].rearrange(
                            "(a p) w -> p a w", p=128
                        ),
                    )
                    e_t = ap_.tile([128, GR * W], f16, name="e_t", tag="e_t", bufs=2)
                    for t, jt in enumerate(jts):
                        nc.vector.tensor_scalar_add(
                            e_t[:, t * W : (t + 1) * W], sb1[:], sdst[:, jt : jt + 1]
                        )
                    l_t = ap_.tile([128, GR * W], f16, name="l_t", tag="l_t", bufs=2)
                    nc.vector.scalar_tensor_tensor(
                        l_t[:, 0:cw], e_t[:, 0:cw], NEG_SLOPE, e_t[:, 0:cw],
                        mybir.AluOpType.mult, mybir.AluOpType.max,
                    )
                    u_t = ap_.tile([128, GR * W], f16, name="u_t", tag="u_t", bufs=3)
                    nc.scalar.activation(
                        u_t[:, 0:cw], l_t[:, 0:cw], mybir.ActivationFunctionType.Exp
                    )
                    p_t = ap_.tile([128, GR * W], f16, name="p_t", tag="p_t", bufs=3)
                    import os as _os
                    if _os.environ.get("GAT_MASK_SPLIT") == "1":
                        h = (cw // 2) // W * W or W
                        nc.vector.tensor_mul(p_t[:, 0:h], adjT[:, 0:h], u_t[:, 0:h])
                        nc.gpsimd.tensor_mul(p_t[:, h:cw], adjT[:, h:cw], u_t[:, h:cw])
                    else:
                        nc.gpsimd.tensor_mul(p_t[:, 0:cw], adjT[:, 0:cw], u_t[:, 0:cw])
                    for t, jt in enumerate(jts):
                        for ib in range(8):
                            nc.tensor.matmul(
                                po[ib][:],
                                p_t[:, t * W + ib * MB : t * W + (ib + 1) * MB],
                                hp[:, jt * 257 : (jt + 1) * 257],
                                start=(jt == 0),
                                stop=(jt == NJT - 1),
                            )

                # ---- phase 2: normalize + store ----
                for ib in range(8):
                    r = fin.tile([MB, 1], f32, name="rcol", tag="rcol")
                    nc.vector.reciprocal(r[:], po[ib][:, 256:257])
                    ob = fin.tile([MB, F], f32, name="ob", tag="ob")
                    nc.vector.scalar_tensor_tensor(
                        ob[:], po[ib][:, 0:F], r[:], wb_sb[0:MB, :],
                        mybir.AluOpType.mult, mybir.AluOpType.add,
                    )
                    nc.sync.dma_start(outd[ib * MB : (ib + 1) * MB, :], ob[:])

    nc.compile()
    return _make_runner(nc, 8)


def _make_runner(nc, n_cores):
    """Compile-once sharded PJRT runner for the 8-core axon path."""
    import time
    import jax
    from jax.sharding import Mesh, PartitionSpec
    from jax.experimental.shard_map import shard_map
    from concourse import mybir
    from concourse.bass2jax import (
        _bass_exec_p,
        install_neuronx_cc_hook,
        partition_id_tensor,
    )

    install_neuronx_cc_hook()
    partition_name = nc.partition_id_tensor.name if nc.partition_id_tensor else None
    in_names, out_names, out_avals, zero_outs = [], [], [], []
    for alloc in nc.m.functions[0].allocations:
        if not isinstance(alloc, mybir.MemoryLocationSet):
            continue
        name = alloc.memorylocations[0].name
        if alloc.kind == "ExternalInput":
            if name != partition_name:
                in_names.append(name)
        elif alloc.kind == "ExternalOutput":
            out_names.append(name)
            shape = tuple(alloc.tensor_shape)
            dtype = mybir.dt.np(alloc.dtype)
            out_avals.append(jax.core.ShapedArray(shape, dtype))
            zero_outs.append(np.zeros(shape, dtype))
    n_params = len(in_names)
    all_in = in_names + out_names + ([partition_name] if partition_name else [])

    def _body(*args):
        operands = list(args)
        if partition_name is not None:
            operands.append(partition_id_tensor())
        return tuple(
            _bass_exec_p.bind(
                *operands,
                out_avals=tuple(out_avals),
                in_names=tuple(all_in),
                out_names=tuple(out_names),
                lowering_input_output_aliases=(),
                sim_require_finite=True,
                sim_require_nnan=True,
                nc=nc,
            )
        )

    devices = jax.devices()[:n_cores]
    mesh = Mesh(np.asarray(devices), ("core",))
    fn = jax.jit(
        shard_map(
            _body,
            mesh=mesh,
            in_specs=(PartitionSpec("core"),) * (n_params + len(out_names)),
            out_specs=(PartitionSpec("core"),) * len(out_names),
            check_rep=False,
        ),
        keep_unused=True,
    )

    def run(in_maps, iters=0):
        per_core = [[np.asarray(m[n]) for n in in_names] for m in in_maps]
        concat_in = [
            np.concatenate([per_core[c][i] for c in range(n_cores)], axis=0)
            for i in range(n_params)
        ]
        concat_zeros = [
            np.zeros((n_cores * z.shape[0], *z.shape[1:]), z.dtype) for z in zero_outs
        ]
        args = [jax.device_put(a) for a in concat_in + concat_zeros]
        out = fn(*args)
        jax.block_until_ready(out)
        times = []
        for _ in range(iters):
            t0 = time.perf_counter()
            out = fn(*args)
            jax.block_until_ready(out)
            times.append(time.perf_counter() - t0)
        results = [
            {
                name: np.asarray(out[i]).reshape(n_cores, *out_avals[i].shape)[c]
                for i, name in enumerate(out_names)
            }
            for c in range(n_cores)
        ]
        return results, (min(times) if times else None)

    return run


def kernel(node_embeddings, adj_matrix, W_w, W_b, a_src, a_dst, a_b):
    global _RUNNER, _last_in_maps
    if _RUNNER is None:
        _RUNNER = _build()

    X = np.asarray(node_embeddings, np.float32)
    adj = np.asarray(adj_matrix, np.int32)
    W_w = np.asarray(W_w, np.float32)
    W_b = np.asarray(W_b, np.float32)
    a_src = np.asarray(a_src, np.float32)
    a_dst = np.asarray(a_dst, np.float32)
    a_b = float(np.asarray(a_b))

    Xb = X.astype(ml_dtypes.bfloat16)
    xtb = np.zeros((F, NP), ml_dtypes.bfloat16)
    xtb[:, :N] = Xb.T
    g_src = (W_w.T @ a_src).astype(np.float32)
    g_dst = (W_w.T @ a_dst).astype(np.float32)
    wg = np.concatenate([W_w.T, g_src[:, None], g_dst[:, None]], axis=1)
    wg = np.ascontiguousarray(wg).astype(ml_dtypes.bfloat16)
    wbt = np.ascontiguousarray(np.tile(W_b, (128, 1))).astype(np.float16)
    cv = np.tile(
        np.array([[float(W_b @ a_src + a_b), float(W_b @ a_dst)]], np.float32),
        (128, 1),
    )

    in_maps = []
    for c in range(8):
        r0 = c * W
        rows = min(W, N - r0)
        adjt = np.zeros((NP, W), np.int32)
        adjt[:N, :rows] = adj[r0 : r0 + rows, :].T
        xwtb = np.zeros((F, W), ml_dtypes.bfloat16)
        xwtb[:, :rows] = Xb[r0 : r0 + rows].T
        in_maps.append(
            {"adjt": adjt, "xtb": xtb, "xwtb": xwtb, "wg": wg, "wbt": wbt, "cv": cv}
        )

    _last_in_maps = in_maps
    results, _ = _RUNNER(in_maps, iters=0)
    out = np.empty((N, F), np.float32)
    for c in range(8):
        r0 = c * W
        rows = min(W, N - r0)
        out[r0 : r0 + rows] = results[c]["out"][:rows]
    return out



# revision 28
# speedup vs baseline: 16.4260x; 16.4260x over previous
"""GAT layer kernel for 8 Trainium2 NeuronCores.

Strategy (row-sharded attention, per the sharding hint):
  - Core c owns query rows [c*1024, (c+1)*1024) of the 8000-node graph
    (1024 = 8*128; core 7's slice is host-padded with zero rows; the key
    axis is padded to 8064 = 63*128 with zero adjacency columns).
  - Each core's adjacency slice is uploaded TRANSPOSED and as int8
    log-space mask bytes {kept: 0, masked: -64} ([8064 keys, 1024
    queries]) so the attention matrix is built directly with keys on
    partitions (the layout the TensorE contraction needs) while
    streaming only 8 MB of HBM per core (vs 32 MB for int32).
  - H' = (X @ W_w^T)^T is computed replicated on every core from a
    host-transposed bf16 X; W_b is folded into the final output (softmax
    rows sum to 1), and all scalar constants fold into the s_src row.
    hp tile layout per key tile: [ones | H'(256) | s_dst], stride 258,
    so one contiguous 257-col slice feeds the matmul (giving the softmax
    denominator in output column 0) and s_dst rides the PSUM drain.
  - Scores in transposed layout: e = s_src[q] (broadcast tile) +
    s_dst[j] (per-partition scalar).  leakyrelu via two 4x-mode
    tensor_scalar ops (e and 0.2*e) + a 2x tensor max on DVE, with a
    configurable fraction of groups instead using ACT's parametric-relu
    (alpha=0.2, bias=s_dst folded) to balance DVE vs ACT load.  exp on
    ACT (no max-subtraction: |e| <~ 3 so exp is safe).
  - The adjacency mask is applied by the cast-DMA itself: the int8
    load uses the DMA compute path (accum_op=add) to do l += {0,-64}
    in flight; exp(l-64) underflows to exactly 0 in fp16, so masked
    weights are exact zeros and unmasked scores are bit-identical
    (adding 0.0 is exact) -- no engine pass for masking at all.
  - attn_unnorm @ [1 | H'] runs as fp16 matmuls accumulating into 8
    PSUM banks; column 0 gives the softmax denominator, applied as a
    reciprocal per-partition multiply (plus the W_b add) while draining.
"""
import os
import sys

sys.path.insert(0, "/opt/trn_rl_repo")

import numpy as np
import ml_dtypes

N, F = 8000, 256
NP = 8064          # padded key count (63 * 128)
W = 1024           # query rows per core (8 * 128; last core mostly padding)
NJT = NP // 128    # 63 key tiles
MB = W // 8        # 128-row output blocks
HPW = 258          # hp stride per key tile: [ones | H'(256) | s_dst]
NEG_SLOPE = 0.2

_RUNNER = None
_last_in_maps = None


def _cfg():
    """Variant knobs (resolved at build time; defaults = best known)."""
    return {
        # which groups use ACT parametric-relu for the leakyrelu
        # (others build e/0.2e on DVE and max them)
        "act_groups": set(
            int(t) for t in os.environ.get("GAT_ACT_GROUPS", "11,13").split(",") if t
        ),
        # lrelu max engine for DVE-mode groups: dve | pool
        "max_eng": os.environ.get("GAT_MAX_ENG", "dve"),
        # mask application:
        #   cceadd - additive log-space mask: the int8 {0,-64} adjacency
        #            DMA accumulates (add) onto the lrelu output; exp of
        #            masked entries underflows to exactly 0 in fp16.
        #   dve / pool - cast {0,1} to f16 and multiply after exp.
        "mask": os.environ.get("GAT_MASK", "cceadd"),
        "gr": int(os.environ.get("GAT_GR", "4")),
        # groups whose 0.2e tensor_scalar runs on gpsimd to relieve DVE
        "pool_e2": set(
            int(t) for t in os.environ.get("GAT_POOL_E2", "1,3,5,7").split(",") if t
        ),
    }


def _build(repeat=1, cfg=None):
    import concourse.bass as bass
    import concourse.tile as tile
    from concourse import bacc, mybir

    if cfg is None:
        cfg = _cfg()
    f16 = mybir.dt.float16
    f32 = mybir.dt.float32
    bf16 = mybir.dt.bfloat16
    A = mybir.AluOpType
    GR = cfg["gr"]

    nc = bacc.Bacc()
    adjtd = nc.dram_tensor("adjt", (NP, W), mybir.dt.int8, kind="ExternalInput")
    xtd = nc.dram_tensor("xtb", (F, NP), bf16, kind="ExternalInput")
    xwtd = nc.dram_tensor("xwtb", (F, W), bf16, kind="ExternalInput")
    wgd = nc.dram_tensor("wg", (F, 257), bf16, kind="ExternalInput")
    gsd = nc.dram_tensor("gs", (F, 2), bf16, kind="ExternalInput")
    wbd = nc.dram_tensor("wbt", (128, F), f16, kind="ExternalInput")
    cvd = nc.dram_tensor("cv", (128, 1), f32, kind="ExternalInput")
    outd = nc.dram_tensor("out", (W, F), f32, kind="ExternalOutput")

    with tile.TileContext(nc) as tc:
        with (
            tc.tile_pool(name="pp", bufs=1) as pp,
            tc.tile_pool(name="att", bufs=2) as ap_,
            tc.tile_pool(name="fin", bufs=2) as fin,
            tc.tile_pool(name="ps", bufs=8, space="PSUM") as psp,
        ):
            for _rep in range(repeat):
                # ---- phase 0a: parameters and transposed activations ----
                # xt in tile-aligned column chunks on the SP queue so early
                # H' tiles can start while later chunks stream; small params
                # go through the Activation HWDGE queue in parallel.
                wg_sb = [pp.tile([128, 257], bf16, name=f"wg{k}", tag=f"wg{k}") for k in range(2)]
                gs_sb = [pp.tile([128, 2], bf16, name=f"gs{k}", tag=f"gs{k}") for k in range(2)]
                xt = [pp.tile([128, NP], bf16, name=f"xt{k}", tag=f"xt{k}") for k in range(2)]
                xwt = [pp.tile([128, W], bf16, name=f"xwt{k}", tag=f"xwt{k}") for k in range(2)]
                for k in range(2):
                    nc.scalar.dma_start(wg_sb[k][:], wgd[k * 128 : (k + 1) * 128, :])
                    nc.scalar.dma_start(gs_sb[k][:], gsd[k * 128 : (k + 1) * 128, :])
                    nc.scalar.dma_start(xwt[k][:], xwtd[k * 128 : (k + 1) * 128, :])
                XCH = [(0, 2048), (2048, 4096), (4096, 6144), (6144, NP)]
                for c0, c1 in XCH:
                    for k in range(2):
                        nc.sync.dma_start(
                            xt[k][:, c0:c1], xtd[k * 128 : (k + 1) * 128, c0:c1]
                        )
                wb_sb = pp.tile([128, F], f16)
                nc.scalar.dma_start(wb_sb[:], wbd[:])
                cv_sb = pp.tile([128, 1], f32)
                nc.scalar.dma_start(cv_sb[:], cvd[:])

                # ---- phase 0b: s_src row (this core's queries) + consts ----
                ssrc_row = pp.tile([1, W], f32)
                for ch in range(2):
                    ps = psp.tile([2, 512], f32, name="ps_s", tag="ps")
                    for k in range(2):
                        nc.tensor.matmul(
                            ps[:],
                            gs_sb[k][:],
                            xwt[k][:, ch * 512 : (ch + 1) * 512],
                            start=(k == 0),
                            stop=(k == 1),
                        )
                    nc.vector.tensor_scalar_add(
                        ssrc_row[0:1, ch * 512 : (ch + 1) * 512], ps[0:1, :], cv_sb[0:1, 0:1]
                    )

                # ---- phase 0c: broadcast s_src across partitions via PE ----
                ones1 = pp.tile([1, 128], f32)
                nc.vector.memset(ones1[:], 1.0)
                sb1 = pp.tile([128, W], f16)
                for ch in range(2):
                    psb_t = psp.tile([128, 512], f32, name="ps_b", tag="ps")
                    nc.tensor.matmul(
                        psb_t[:], ones1[:], ssrc_row[0:1, ch * 512 : (ch + 1) * 512],
                        start=True, stop=True,
                    )
                    nc.vector.tensor_copy(sb1[:, ch * 512 : (ch + 1) * 512], psb_t[:])

                # ---- phase 0d: H' key tiles ([ones | H | s_dst] per tile) ----
                hp = pp.tile([128, NJT * HPW], f16)
                hp3 = hp[:].rearrange("p (j c) -> p j c", c=HPW)
                nc.vector.memset(hp3[:, :, 0:1], 1.0)
                for jt in range(NJT):
                    ph = psp.tile([128, 257], f32, name="ps_h", tag="ps")
                    for k in range(2):
                        nc.tensor.matmul(
                            ph[:],
                            xt[k][:, jt * 128 : (jt + 1) * 128],
                            wg_sb[k][:],
                            start=(k == 0),
                            stop=(k == 1),
                        )
                    # H' -> cols 1..256, s_dst_raw -> col 257
                    if jt % 2 == 0:
                        nc.vector.tensor_copy(hp[:, jt * HPW + 1 : jt * HPW + 258], ph[:])
                    else:
                        nc.scalar.copy(hp[:, jt * HPW + 1 : jt * HPW + 258], ph[:])

                # dense f32 staging of the s_dst per-partition scalars
                sdst = pp.tile([128, NJT], f32)

                # ---- phase 1: masked attention weights + matmul accumulate ----
                po = [psp.tile([128, 257], f32, name=f"po{ib}", tag="ps") for ib in range(8)]
                groups = [list(range(g, min(g + GR, NJT))) for g in range(0, NJT, GR)]
                for gi, jts in enumerate(groups):
                    na = len(jts)
                    cw = na * W
                    j0 = jts[0]
                    nc.vector.tensor_copy(
                        sdst[:].rearrange("p (j c) -> p j c", c=1)[:, j0 : j0 + na, :],
                        hp3[:, j0 : j0 + na, 257:258],
                    )
                    adj_src = adjtd[j0 * 128 : (j0 + na) * 128, :].rearrange(
                        "(a p) w -> p a w", p=128
                    )
                    u_t = ap_.tile([128, GR * W], f16, name="u_t", tag="u_t", bufs=3)
                    if gi in cfg["act_groups"]:
                        # ACT path: l = prelu(sb1 + s_dst) in one pass per tile
                        l_t = ap_.tile([128, GR * W], f16, name="l_t", tag="l_t", bufs=4)
                        for t, jt in enumerate(jts):
                            nc.scalar.activation(
                                l_t[:, t * W : (t + 1) * W], sb1[:],
                                mybir.ActivationFunctionType.Prelu,
                                bias=sdst[:, jt : jt + 1], scale=1.0, alpha=NEG_SLOPE,
                            )
                    else:
                        # DVE path: e, 0.2e via 4x-mode tensor_scalar, then max
                        e_t = ap_.tile([128, GR * W], f16, name="e_t", tag="e_t", bufs=2)
                        l_t = ap_.tile([128, GR * W], f16, name="l_t", tag="l_t", bufs=4)
                        e2eng = nc.gpsimd if gi in cfg["pool_e2"] else nc.vector
                        for t, jt in enumerate(jts):
                            nc.vector.tensor_scalar_add(
                                e_t[:, t * W : (t + 1) * W], sb1[:], sdst[:, jt : jt + 1]
                            )
                            e2eng.tensor_scalar(
                                l_t[:, t * W : (t + 1) * W], sb1[:],
                                sdst[:, jt : jt + 1], NEG_SLOPE, A.add, A.mult,
                            )
                        if cfg["max_eng"] == "pool":
                            nc.gpsimd.tensor_max(l_t[:, 0:cw], e_t[:, 0:cw], l_t[:, 0:cw])
                        elif cfg["max_eng"] == "split":
                            h = (cw // 2) // W * W or W
                            nc.vector.tensor_max(l_t[:, 0:h], e_t[:, 0:h], l_t[:, 0:h])
                            nc.gpsimd.tensor_max(l_t[:, h:cw], e_t[:, h:cw], l_t[:, h:cw])
                        else:
                            nc.vector.tensor_max(l_t[:, 0:cw], e_t[:, 0:cw], l_t[:, 0:cw])
                    if cfg["mask"] == "cceadd":
                        # additive mask in log space: l += {0,-64}; exp -> 0
                        nc.gpsimd.dma_start(
                            l_t[:].rearrange("p (a w) -> p a w", w=W)[:, 0:na, :],
                            adj_src,
                            accum_op=A.add,
                        )
                        nc.scalar.activation(
                            u_t[:, 0:cw], l_t[:, 0:cw], mybir.ActivationFunctionType.Exp
                        )
                        p_t = u_t
                    else:
                        nc.scalar.activation(
                            u_t[:, 0:cw], l_t[:, 0:cw], mybir.ActivationFunctionType.Exp
                        )
                        adjT = ap_.tile([128, GR * W], f16, name="adjT", tag="adjT", bufs=3)
                        nc.gpsimd.dma_start(
                            adjT[:].rearrange("p (a w) -> p a w", w=W)[:, 0:na, :],
                            adj_src,
                        )
                        p_t = ap_.tile([128, GR * W], f16, name="p_t", tag="p_t", bufs=3)
                        if cfg["mask"] == "pool":
                            nc.gpsimd.tensor_mul(p_t[:, 0:cw], adjT[:, 0:cw], u_t[:, 0:cw])
                        else:
                            nc.vector.tensor_mul(p_t[:, 0:cw], adjT[:, 0:cw], u_t[:, 0:cw])
                    for t, jt in enumerate(jts):
                        for ib in range(8):
                            nc.tensor.matmul(
                                po[ib][:],
                                p_t[:, t * W + ib * MB : t * W + (ib + 1) * MB],
                                hp[:, jt * HPW : jt * HPW + 257],
                                start=(jt == 0),
                                stop=(jt == NJT - 1),
                            )

                # ---- phase 2: normalize + store ----
                for ib in range(8):
                    r = fin.tile([MB, 1], f32, name="rcol", tag="rcol")
                    nc.vector.reciprocal(r[:], po[ib][:, 0:1])
                    ob = fin.tile([MB, F], f32, name="ob", tag="ob")
                    nc.vector.scalar_tensor_tensor(
                        ob[:], po[ib][:, 1:257], r[:], wb_sb[0:MB, :],
                        A.mult, A.add,
                    )
                    nc.sync.dma_start(outd[ib * MB : (ib + 1) * MB, :], ob[:])

    nc.compile()
    return _make_runner(nc, 8)


def _make_runner(nc, n_cores):
    """Compile-once sharded PJRT runner for the 8-core axon path."""
    import time
    import jax
    from jax.sharding import Mesh, PartitionSpec
    from jax.experimental.shard_map import shard_map
    from concourse import mybir
    from concourse.bass2jax import (
        _bass_exec_p,
        install_neuronx_cc_hook,
        partition_id_tensor,
    )

    install_neuronx_cc_hook()
    partition_name = nc.partition_id_tensor.name if nc.partition_id_tensor else None
    in_names, out_names, out_avals, zero_outs = [], [], [], []
    for alloc in nc.m.functions[0].allocations:
        if not isinstance(alloc, mybir.MemoryLocationSet):
            continue
        name = alloc.memorylocations[0].name
        if alloc.kind == "ExternalInput":
            if name != partition_name:
                in_names.append(name)
        elif alloc.kind == "ExternalOutput":
            out_names.append(name)
            shape = tuple(alloc.tensor_shape)
            dtype = mybir.dt.np(alloc.dtype)
            out_avals.append(jax.core.ShapedArray(shape, dtype))
            zero_outs.append(np.zeros(shape, dtype))
    n_params = len(in_names)
    all_in = in_names + out_names + ([partition_name] if partition_name else [])

    def _body(*args):
        operands = list(args)
        if partition_name is not None:
            operands.append(partition_id_tensor())
        return tuple(
            _bass_exec_p.bind(
                *operands,
                out_avals=tuple(out_avals),
                in_names=tuple(all_in),
                out_names=tuple(out_names),
                lowering_input_output_aliases=(),
                sim_require_finite=True,
                sim_require_nnan=True,
                nc=nc,
            )
        )

    devices = jax.devices()[:n_cores]
    mesh = Mesh(np.asarray(devices), ("core",))
    fn = jax.jit(
        shard_map(
            _body,
            mesh=mesh,
            in_specs=(PartitionSpec("core"),) * (n_params + len(out_names)),
            out_specs=(PartitionSpec("core"),) * len(out_names),
            check_rep=False,
        ),
        keep_unused=True,
    )

    def run(in_maps, iters=0):
        per_core = [[np.asarray(m[n]) for n in in_names] for m in in_maps]
        concat_in = [
            np.concatenate([per_core[c][i] for c in range(n_cores)], axis=0)
            for i in range(n_params)
        ]
        concat_zeros = [
            np.zeros((n_cores * z.shape[0], *z.shape[1:]), z.dtype) for z in zero_outs
        ]
        args = [jax.device_put(a) for a in concat_in + concat_zeros]
        out = fn(*args)
        jax.block_until_ready(out)
        times = []
        for _ in range(iters):
            t0 = time.perf_counter()
            out = fn(*args)
            jax.block_until_ready(out)
            times.append(time.perf_counter() - t0)
        results = [
            {
                name: np.asarray(out[i]).reshape(n_cores, *out_avals[i].shape)[c]
                for i, name in enumerate(out_names)
            }
            for c in range(n_cores)
        ]
        return results, (min(times) if times else None)

    return run


def _prep_in_maps(node_embeddings, adj_matrix, W_w, W_b, a_src, a_dst, a_b, mask="cceadd"):
    X = np.asarray(node_embeddings, np.float32)
    adj = np.asarray(adj_matrix, np.int32)
    W_w = np.asarray(W_w, np.float32)
    W_b = np.asarray(W_b, np.float32)
    a_src = np.asarray(a_src, np.float32)
    a_dst = np.asarray(a_dst, np.float32)
    a_b = float(np.asarray(a_b))

    Xb = X.astype(ml_dtypes.bfloat16)
    xtb = np.zeros((F, NP), ml_dtypes.bfloat16)
    xtb[:, :N] = Xb.T
    g_src = (W_w.T @ a_src).astype(np.float32)
    g_dst = (W_w.T @ a_dst).astype(np.float32)
    wg = np.concatenate([W_w.T, g_dst[:, None]], axis=1)
    wg = np.ascontiguousarray(wg).astype(ml_dtypes.bfloat16)
    gs = np.ascontiguousarray(
        np.concatenate([g_src[:, None], g_dst[:, None]], axis=1)
    ).astype(ml_dtypes.bfloat16)
    wbt = np.ascontiguousarray(np.tile(W_b, (128, 1))).astype(np.float16)
    cconst = float(W_b @ a_src + W_b @ a_dst + a_b)
    cv = np.full((128, 1), cconst, np.float32)

    in_maps = []
    for c in range(8):
        r0 = c * W
        rows = min(W, N - r0)
        if mask == "cceadd":
            # {kept: 0, masked: -64}; padding stays masked
            adjt = np.full((NP, W), -64, np.int8)
            adjt[:N, :rows] = (adj[r0 : r0 + rows, :].T.astype(np.int16) - 1).astype(
                np.int8
            ) * 64
        else:
            adjt = np.zeros((NP, W), np.int8)
            adjt[:N, :rows] = adj[r0 : r0 + rows, :].T.astype(np.int8)
        xwtb = np.zeros((F, W), ml_dtypes.bfloat16)
        xwtb[:, :rows] = Xb[r0 : r0 + rows].T
        in_maps.append(
            {
                "adjt": adjt,
                "xtb": xtb,
                "xwtb": xwtb,
                "wg": wg,
                "gs": gs,
                "wbt": wbt,
                "cv": cv,
            }
        )
    return in_maps


def kernel(node_embeddings, adj_matrix, W_w, W_b, a_src, a_dst, a_b):
    global _RUNNER, _last_in_maps
    if _RUNNER is None:
        _RUNNER = _build()

    in_maps = _prep_in_maps(
        node_embeddings, adj_matrix, W_w, W_b, a_src, a_dst, a_b, mask=_cfg()["mask"]
    )
    _last_in_maps = in_maps
    results, _ = _RUNNER(in_maps, iters=0)
    out = np.empty((N, F), np.float32)
    for c in range(8):
        r0 = c * W
        rows = min(W, N - r0)
        out[r0 : r0 + rows] = results[c]["out"][:rows]
    return out


# revision 29
# speedup vs baseline: 25.5767x; 1.5571x over previous
"""GAT layer kernel for 8 Trainium2 NeuronCores.

Strategy (row-sharded attention, per the sharding hint):
  - Core c owns query rows [c*1024, (c+1)*1024) of the 8000-node graph
    (1024 = 8*128; core 7's slice is host-padded with zero rows; the key
    axis is padded to 8064 = 63*128 with zero adjacency columns).
  - Each core's adjacency slice is uploaded TRANSPOSED and as int8
    log-space mask bytes {kept: 0, masked: -64} ([8064 keys, 1024
    queries]) so the attention matrix is built directly with keys on
    partitions (the layout the TensorE contraction needs) while
    streaming only 8 MB of HBM per core (vs 32 MB for int32).
  - H' = (X @ W_w^T)^T is computed replicated on every core from a
    host-transposed bf16 X; W_b is folded into the final output (softmax
    rows sum to 1), and all scalar constants fold into the s_src row.
    hp tile layout per key tile: [ones | H'(256) | s_dst], stride 258,
    so one contiguous 257-col slice feeds the matmul (giving the softmax
    denominator in output column 0) and s_dst rides the PSUM drain.
  - Scores in transposed layout: e = s_src[q] (broadcast tile) +
    s_dst[j] (per-partition scalar).  leakyrelu via two 4x-mode
    tensor_scalar ops (e and 0.2*e) + a 2x tensor max on DVE, with a
    configurable fraction of groups instead using ACT's parametric-relu
    (alpha=0.2, bias=s_dst folded) to balance DVE vs ACT load.  exp on
    ACT (no max-subtraction: |e| <~ 3 so exp is safe).
  - The adjacency mask is applied by the cast-DMA itself: the int8
    load uses the DMA compute path (accum_op=add) to do l += {0,-64}
    in flight; exp(l-64) underflows to exactly 0 in fp16, so masked
    weights are exact zeros and unmasked scores are bit-identical
    (adding 0.0 is exact) -- no engine pass for masking at all.
  - attn_unnorm @ [1 | H'] runs as fp16 matmuls accumulating into 8
    PSUM banks; column 0 gives the softmax denominator, applied as a
    reciprocal per-partition multiply (plus the W_b add) while draining.
"""
import os
import sys

sys.path.insert(0, "/opt/trn_rl_repo")

import numpy as np
import ml_dtypes

N, F = 8000, 256
NP = 8064          # padded key count (63 * 128)
W = 1024           # query rows per core (8 * 128; last core mostly padding)
NJT = NP // 128    # 63 key tiles
MB = W // 8        # 128-row output blocks
HPW = 258          # hp stride per key tile: [ones | H'(256) | s_dst]
NEG_SLOPE = 0.2

_RUNNER = None
_last_in_maps = None


def _cfg():
    """Variant knobs (resolved at build time; defaults = best known)."""
    return {
        # which groups use ACT parametric-relu for the leakyrelu
        # (others build e/0.2e on DVE and max them)
        "act_groups": set(
            int(t) for t in os.environ.get("GAT_ACT_GROUPS", "11,13").split(",") if t
        ),
        # lrelu max engine for DVE-mode groups: dve | pool
        "max_eng": os.environ.get("GAT_MAX_ENG", "dve"),
        # mask application:
        #   cceadd - additive log-space mask: the int8 {0,-64} adjacency
        #            DMA accumulates (add) onto the lrelu output; exp of
        #            masked entries underflows to exactly 0 in fp16.
        #   dve / pool - cast {0,1} to f16 and multiply after exp.
        "mask": os.environ.get("GAT_MASK", "cceadd"),
        "gr": int(os.environ.get("GAT_GR", "4")),
        # groups whose 0.2e tensor_scalar runs on gpsimd to relieve DVE
        # (lower max-engine busy but longer critical path; default off)
        "pool_e2": set(
            int(t) for t in os.environ.get("GAT_POOL_E2", "").split(",") if t
        ),
    }


def _build(repeat=1, cfg=None):
    import concourse.bass as bass
    import concourse.tile as tile
    from concourse import bacc, mybir

    if cfg is None:
        cfg = _cfg()
    f16 = mybir.dt.float16
    f32 = mybir.dt.float32
    bf16 = mybir.dt.bfloat16
    A = mybir.AluOpType
    GR = cfg["gr"]

    nc = bacc.Bacc()
    adjtd = nc.dram_tensor("adjt", (NP, W), mybir.dt.int8, kind="ExternalInput")
    xtd = nc.dram_tensor("xtb", (F, NP), bf16, kind="ExternalInput")
    xwtd = nc.dram_tensor("xwtb", (F, W), bf16, kind="ExternalInput")
    wgd = nc.dram_tensor("wg", (F, 257), bf16, kind="ExternalInput")
    gsd = nc.dram_tensor("gs", (F, 2), bf16, kind="ExternalInput")
    wbd = nc.dram_tensor("wbt", (128, F), f16, kind="ExternalInput")
    cvd = nc.dram_tensor("cv", (128, 1), f32, kind="ExternalInput")
    outd = nc.dram_tensor("out", (W, F), f32, kind="ExternalOutput")

    with tile.TileContext(nc) as tc:
        with (
            tc.tile_pool(name="pp", bufs=1) as pp,
            tc.tile_pool(name="att", bufs=2) as ap_,
            tc.tile_pool(name="fin", bufs=2) as fin,
            tc.tile_pool(name="ps", bufs=8, space="PSUM") as psp,
        ):
            for _rep in range(repeat):
                # ---- phase 0a: parameters and transposed activations ----
                # xt in tile-aligned column chunks on the SP queue so early
                # H' tiles can start while later chunks stream; small params
                # go through the Activation HWDGE queue in parallel.
                wg_sb = [pp.tile([128, 257], bf16, name=f"wg{k}", tag=f"wg{k}") for k in range(2)]
                gs_sb = [pp.tile([128, 2], bf16, name=f"gs{k}", tag=f"gs{k}") for k in range(2)]
                xt = [pp.tile([128, NP], bf16, name=f"xt{k}", tag=f"xt{k}") for k in range(2)]
                xwt = [pp.tile([128, W], bf16, name=f"xwt{k}", tag=f"xwt{k}") for k in range(2)]
                for k in range(2):
                    nc.scalar.dma_start(wg_sb[k][:], wgd[k * 128 : (k + 1) * 128, :])
                    nc.scalar.dma_start(gs_sb[k][:], gsd[k * 128 : (k + 1) * 128, :])
                    nc.scalar.dma_start(xwt[k][:], xwtd[k * 128 : (k + 1) * 128, :])
                XCH = [(0, 2048), (2048, 4096), (4096, 6144), (6144, NP)]
                for c0, c1 in XCH:
                    for k in range(2):
                        nc.sync.dma_start(
                            xt[k][:, c0:c1], xtd[k * 128 : (k + 1) * 128, c0:c1]
                        )
                wb_sb = pp.tile([128, F], f16)
                nc.scalar.dma_start(wb_sb[:], wbd[:])
                cv_sb = pp.tile([128, 1], f32)
                nc.scalar.dma_start(cv_sb[:], cvd[:])

                # ---- phase 0b: s_src row (this core's queries) + consts ----
                ssrc_row = pp.tile([1, W], f32)
                for ch in range(2):
                    ps = psp.tile([2, 512], f32, name="ps_s", tag="ps")
                    for k in range(2):
                        nc.tensor.matmul(
                            ps[:],
                            gs_sb[k][:],
                            xwt[k][:, ch * 512 : (ch + 1) * 512],
                            start=(k == 0),
                            stop=(k == 1),
                        )
                    nc.vector.tensor_scalar_add(
                        ssrc_row[0:1, ch * 512 : (ch + 1) * 512], ps[0:1, :], cv_sb[0:1, 0:1]
                    )

                # ---- phase 0c: broadcast s_src across partitions via PE ----
                ones1 = pp.tile([1, 128], f32)
                nc.vector.memset(ones1[:], 1.0)
                sb1 = pp.tile([128, W], f16)
                for ch in range(2):
                    psb_t = psp.tile([128, 512], f32, name="ps_b", tag="ps")
                    nc.tensor.matmul(
                        psb_t[:], ones1[:], ssrc_row[0:1, ch * 512 : (ch + 1) * 512],
                        start=True, stop=True,
                    )
                    nc.vector.tensor_copy(sb1[:, ch * 512 : (ch + 1) * 512], psb_t[:])

                # ---- phase 0d: H' key tiles ([ones | H | s_dst] per tile) ----
                hp = pp.tile([128, NJT * HPW], f16)
                hp3 = hp[:].rearrange("p (j c) -> p j c", c=HPW)
                nc.vector.memset(hp3[:, :, 0:1], 1.0)
                for jt in range(NJT):
                    ph = psp.tile([128, 257], f32, name="ps_h", tag="ps")
                    for k in range(2):
                        nc.tensor.matmul(
                            ph[:],
                            xt[k][:, jt * 128 : (jt + 1) * 128],
                            wg_sb[k][:],
                            start=(k == 0),
                            stop=(k == 1),
                        )
                    # H' -> cols 1..256, s_dst_raw -> col 257
                    if jt % 2 == 0:
                        nc.vector.tensor_copy(hp[:, jt * HPW + 1 : jt * HPW + 258], ph[:])
                    else:
                        nc.scalar.copy(hp[:, jt * HPW + 1 : jt * HPW + 258], ph[:])

                # dense f32 staging of the s_dst per-partition scalars
                sdst = pp.tile([128, NJT], f32)

                # ---- phase 1: masked attention weights + matmul accumulate ----
                po = [psp.tile([128, 257], f32, name=f"po{ib}", tag="ps") for ib in range(8)]
                groups = [list(range(g, min(g + GR, NJT))) for g in range(0, NJT, GR)]
                for gi, jts in enumerate(groups):
                    na = len(jts)
                    cw = na * W
                    j0 = jts[0]
                    nc.vector.tensor_copy(
                        sdst[:].rearrange("p (j c) -> p j c", c=1)[:, j0 : j0 + na, :],
                        hp3[:, j0 : j0 + na, 257:258],
                    )
                    adj_src = adjtd[j0 * 128 : (j0 + na) * 128, :].rearrange(
                        "(a p) w -> p a w", p=128
                    )
                    u_t = ap_.tile([128, GR * W], f16, name="u_t", tag="u_t", bufs=3)
                    if gi in cfg["act_groups"]:
                        # ACT path: l = prelu(sb1 + s_dst) in one pass per tile
                        l_t = ap_.tile([128, GR * W], f16, name="l_t", tag="l_t", bufs=4)
                        for t, jt in enumerate(jts):
                            nc.scalar.activation(
                                l_t[:, t * W : (t + 1) * W], sb1[:],
                                mybir.ActivationFunctionType.Prelu,
                                bias=sdst[:, jt : jt + 1], scale=1.0, alpha=NEG_SLOPE,
                            )
                    else:
                        # DVE path: e, 0.2e via 4x-mode tensor_scalar, then max
                        e_t = ap_.tile([128, GR * W], f16, name="e_t", tag="e_t", bufs=2)
                        l_t = ap_.tile([128, GR * W], f16, name="l_t", tag="l_t", bufs=4)
                        e2eng = nc.gpsimd if gi in cfg["pool_e2"] else nc.vector
                        for t, jt in enumerate(jts):
                            nc.vector.tensor_scalar_add(
                                e_t[:, t * W : (t + 1) * W], sb1[:], sdst[:, jt : jt + 1]
                            )
                            e2eng.tensor_scalar(
                                l_t[:, t * W : (t + 1) * W], sb1[:],
                                sdst[:, jt : jt + 1], NEG_SLOPE, A.add, A.mult,
                            )
                        if cfg["max_eng"] == "pool":
                            nc.gpsimd.tensor_max(l_t[:, 0:cw], e_t[:, 0:cw], l_t[:, 0:cw])
                        elif cfg["max_eng"] == "split":
                            h = (cw // 2) // W * W or W
                            nc.vector.tensor_max(l_t[:, 0:h], e_t[:, 0:h], l_t[:, 0:h])
                            nc.gpsimd.tensor_max(l_t[:, h:cw], e_t[:, h:cw], l_t[:, h:cw])
                        else:
                            nc.vector.tensor_max(l_t[:, 0:cw], e_t[:, 0:cw], l_t[:, 0:cw])
                    if cfg["mask"] == "cceadd":
                        # additive mask in log space: l += {0,-64}; exp -> 0
                        nc.gpsimd.dma_start(
                            l_t[:].rearrange("p (a w) -> p a w", w=W)[:, 0:na, :],
                            adj_src,
                            accum_op=A.add,
                        )
                        nc.scalar.activation(
                            u_t[:, 0:cw], l_t[:, 0:cw], mybir.ActivationFunctionType.Exp
                        )
                        p_t = u_t
                    else:
                        nc.scalar.activation(
                            u_t[:, 0:cw], l_t[:, 0:cw], mybir.ActivationFunctionType.Exp
                        )
                        adjT = ap_.tile([128, GR * W], f16, name="adjT", tag="adjT", bufs=3)
                        nc.gpsimd.dma_start(
                            adjT[:].rearrange("p (a w) -> p a w", w=W)[:, 0:na, :],
                            adj_src,
                        )
                        p_t = ap_.tile([128, GR * W], f16, name="p_t", tag="p_t", bufs=3)
                        if cfg["mask"] == "pool":
                            nc.gpsimd.tensor_mul(p_t[:, 0:cw], adjT[:, 0:cw], u_t[:, 0:cw])
                        else:
                            nc.vector.tensor_mul(p_t[:, 0:cw], adjT[:, 0:cw], u_t[:, 0:cw])
                    for t, jt in enumerate(jts):
                        for ib in range(8):
                            nc.tensor.matmul(
                                po[ib][:],
                                p_t[:, t * W + ib * MB : t * W + (ib + 1) * MB],
                                hp[:, jt * HPW : jt * HPW + 257],
                                start=(jt == 0),
                                stop=(jt == NJT - 1),
                            )

                # ---- phase 2: normalize + store ----
                for ib in range(8):
                    r = fin.tile([MB, 1], f32, name="rcol", tag="rcol")
                    nc.vector.reciprocal(r[:], po[ib][:, 0:1])
                    ob = fin.tile([MB, F], f32, name="ob", tag="ob")
                    nc.vector.scalar_tensor_tensor(
                        ob[:], po[ib][:, 1:257], r[:], wb_sb[0:MB, :],
                        A.mult, A.add,
                    )
                    nc.sync.dma_start(outd[ib * MB : (ib + 1) * MB, :], ob[:])

    nc.compile()
    return _make_runner(nc, 8)


def _make_runner(nc, n_cores):
    """Compile-once sharded PJRT runner for the 8-core axon path."""
    import time
    import jax
    from jax.sharding import Mesh, PartitionSpec
    from jax.experimental.shard_map import shard_map
    from concourse import mybir
    from concourse.bass2jax import (
        _bass_exec_p,
        install_neuronx_cc_hook,
        partition_id_tensor,
    )

    install_neuronx_cc_hook()
    partition_name = nc.partition_id_tensor.name if nc.partition_id_tensor else None
    in_names, out_names, out_avals, zero_outs = [], [], [], []
    for alloc in nc.m.functions[0].allocations:
        if not isinstance(alloc, mybir.MemoryLocationSet):
            continue
        name = alloc.memorylocations[0].name
        if alloc.kind == "ExternalInput":
            if name != partition_name:
                in_names.append(name)
        elif alloc.kind == "ExternalOutput":
            out_names.append(name)
            shape = tuple(alloc.tensor_shape)
            dtype = mybir.dt.np(alloc.dtype)
            out_avals.append(jax.core.ShapedArray(shape, dtype))
            zero_outs.append(np.zeros(shape, dtype))
    n_params = len(in_names)
    all_in = in_names + out_names + ([partition_name] if partition_name else [])

    def _body(*args):
        operands = list(args)
        if partition_name is not None:
            operands.append(partition_id_tensor())
        return tuple(
            _bass_exec_p.bind(
                *operands,
                out_avals=tuple(out_avals),
                in_names=tuple(all_in),
                out_names=tuple(out_names),
                lowering_input_output_aliases=(),
                sim_require_finite=True,
                sim_require_nnan=True,
                nc=nc,
            )
        )

    devices = jax.devices()[:n_cores]
    mesh = Mesh(np.asarray(devices), ("core",))
    fn = jax.jit(
        shard_map(
            _body,
            mesh=mesh,
            in_specs=(PartitionSpec("core"),) * (n_params + len(out_names)),
            out_specs=(PartitionSpec("core"),) * len(out_names),
            check_rep=False,
        ),
        keep_unused=True,
    )

    def run(in_maps, iters=0):
        per_core = [[np.asarray(m[n]) for n in in_names] for m in in_maps]
        concat_in = [
            np.concatenate([per_core[c][i] for c in range(n_cores)], axis=0)
            for i in range(n_params)
        ]
        concat_zeros = [
            np.zeros((n_cores * z.shape[0], *z.shape[1:]), z.dtype) for z in zero_outs
        ]
        args = [jax.device_put(a) for a in concat_in + concat_zeros]
        out = fn(*args)
        jax.block_until_ready(out)
        times = []
        for _ in range(iters):
            t0 = time.perf_counter()
            out = fn(*args)
            jax.block_until_ready(out)
            times.append(time.perf_counter() - t0)
        results = [
            {
                name: np.asarray(out[i]).reshape(n_cores, *out_avals[i].shape)[c]
                for i, name in enumerate(out_names)
            }
            for c in range(n_cores)
        ]
        return results, (min(times) if times else None)

    return run


def _prep_in_maps(node_embeddings, adj_matrix, W_w, W_b, a_src, a_dst, a_b, mask="cceadd"):
    X = np.asarray(node_embeddings, np.float32)
    adj = np.asarray(adj_matrix, np.int32)
    W_w = np.asarray(W_w, np.float32)
    W_b = np.asarray(W_b, np.float32)
    a_src = np.asarray(a_src, np.float32)
    a_dst = np.asarray(a_dst, np.float32)
    a_b = float(np.asarray(a_b))

    Xb = X.astype(ml_dtypes.bfloat16)
    xtb = np.zeros((F, NP), ml_dtypes.bfloat16)
    xtb[:, :N] = Xb.T
    g_src = (W_w.T @ a_src).astype(np.float32)
    g_dst = (W_w.T @ a_dst).astype(np.float32)
    wg = np.concatenate([W_w.T, g_dst[:, None]], axis=1)
    wg = np.ascontiguousarray(wg).astype(ml_dtypes.bfloat16)
    gs = np.ascontiguousarray(
        np.concatenate([g_src[:, None], g_dst[:, None]], axis=1)
    ).astype(ml_dtypes.bfloat16)
    wbt = np.ascontiguousarray(np.tile(W_b, (128, 1))).astype(np.float16)
    cconst = float(W_b @ a_src + W_b @ a_dst + a_b)
    cv = np.full((128, 1), cconst, np.float32)

    in_maps = []
    for c in range(8):
        r0 = c * W
        rows = min(W, N - r0)
        if mask == "cceadd":
            # {kept: 0, masked: -64}; padding stays masked
            adjt = np.full((NP, W), -64, np.int8)
            adjt[:N, :rows] = (adj[r0 : r0 + rows, :].T.astype(np.int16) - 1).astype(
                np.int8
            ) * 64
        else:
            adjt = np.zeros((NP, W), np.int8)
            adjt[:N, :rows] = adj[r0 : r0 + rows, :].T.astype(np.int8)
        xwtb = np.zeros((F, W), ml_dtypes.bfloat16)
        xwtb[:, :rows] = Xb[r0 : r0 + rows].T
        in_maps.append(
            {
                "adjt": adjt,
                "xtb": xtb,
                "xwtb": xwtb,
                "wg": wg,
                "gs": gs,
                "wbt": wbt,
                "cv": cv,
            }
        )
    return in_maps


def kernel(node_embeddings, adj_matrix, W_w, W_b, a_src, a_dst, a_b):
    global _RUNNER, _last_in_maps
    if _RUNNER is None:
        _RUNNER = _build()

    in_maps = _prep_in_maps(
        node_embeddings, adj_matrix, W_w, W_b, a_src, a_dst, a_b, mask=_cfg()["mask"]
    )
    _last_in_maps = in_maps
    results, _ = _RUNNER(in_maps, iters=0)
    out = np.empty((N, F), np.float32)
    for c in range(8):
        r0 = c * W
        rows = min(W, N - r0)
        out[r0 : r0 + rows] = results[c]["out"][:rows]
    return out


# revision 30
# speedup vs baseline: 32.0538x; 1.2532x over previous
"""GAT layer kernel for 8 Trainium2 NeuronCores.

Strategy (row-sharded attention, per the sharding hint):
  - Core c owns query rows [c*1024, (c+1)*1024) of the 8000-node graph
    (1024 = 8*128; core 7's slice is host-padded with zero rows; the key
    axis is padded to 8064 = 63*128 with zero adjacency columns).
  - Each core's adjacency slice is uploaded TRANSPOSED and as int8
    log-space mask bytes {kept: 0, masked: -64} ([8064 keys, 1024
    queries]) so the attention matrix is built directly with keys on
    partitions (the layout the TensorE contraction needs) while
    streaming only 8 MB of HBM per core (vs 32 MB for int32).
  - H' = (X @ W_w^T)^T is computed replicated on every core from a
    host-transposed bf16 X; W_b is folded into the final output (softmax
    rows sum to 1), and all scalar constants fold into the s_src row.
    hp tile layout per key tile: [ones | H'(256) | s_dst], stride 258,
    so one contiguous 257-col slice feeds the matmul (giving the softmax
    denominator in output column 0) and s_dst rides the PSUM drain.
  - Scores in transposed layout: e = s_src[q] (broadcast tile) +
    s_dst[j] (per-partition scalar).  leakyrelu via two 4x-mode
    tensor_scalar ops (e and 0.2*e) + a 2x tensor max on DVE, with a
    configurable fraction of groups instead using ACT's parametric-relu
    (alpha=0.2, bias=s_dst folded) to balance DVE vs ACT load.  exp on
    ACT (no max-subtraction: |e| <~ 3 so exp is safe).
  - The adjacency mask is applied by the cast-DMA itself: the int8
    load uses the DMA compute path (accum_op=add) to do l += {0,-64}
    in flight; exp(l-64) underflows to exactly 0 in fp16, so masked
    weights are exact zeros and unmasked scores are bit-identical
    (adding 0.0 is exact) -- no engine pass for masking at all.
  - attn_unnorm @ [1 | H'] runs as fp16 matmuls accumulating into 8
    PSUM banks; column 0 gives the softmax denominator, applied as a
    reciprocal per-partition multiply (plus the W_b add) while draining.
"""
import os
import sys

sys.path.insert(0, "/opt/trn_rl_repo")

import numpy as np
import ml_dtypes

N, F = 8000, 256
NP = 8064          # padded key count (63 * 128)
W = 1024           # query rows per core (8 * 128; last core mostly padding)
NJT = NP // 128    # 63 key tiles
MB = W // 8        # 128-row output blocks
HPW = 258          # hp stride per key tile: [ones | H'(256) | s_dst]
NEG_SLOPE = 0.2

_RUNNER = None
_last_in_maps = None


def _cfg():
    """Variant knobs (resolved at build time; defaults = best known)."""
    return {
        # which groups use ACT parametric-relu for the leakyrelu
        # (others build e/0.2e on DVE and max them)
        # early placement fills ACT's initial idle window (it otherwise
        # waits on the first DVE lrelu chains) and shortens its tail
        "act_groups": set(
            int(t) for t in os.environ.get("GAT_ACT_GROUPS", "2,5").split(",") if t
        ),
        # lrelu max engine for DVE-mode groups: dve | pool
        "max_eng": os.environ.get("GAT_MAX_ENG", "dve"),
        # mask application:
        #   cceadd - additive log-space mask: the int8 {0,-64} adjacency
        #            DMA accumulates (add) onto the lrelu output; exp of
        #            masked entries underflows to exactly 0 in fp16.
        #   dve / pool - cast {0,1} to f16 and multiply after exp.
        "mask": os.environ.get("GAT_MASK", "cceadd"),
        "gr": int(os.environ.get("GAT_GR", "4")),
        # groups whose 0.2e tensor_scalar runs on gpsimd to relieve DVE
        # (lower max-engine busy but longer critical path; default off)
        "pool_e2": set(
            int(t) for t in os.environ.get("GAT_POOL_E2", "").split(",") if t
        ),
    }


def _build(repeat=1, cfg=None):
    import concourse.bass as bass
    import concourse.tile as tile
    from concourse import bacc, mybir

    if cfg is None:
        cfg = _cfg()
    f16 = mybir.dt.float16
    f32 = mybir.dt.float32
    bf16 = mybir.dt.bfloat16
    A = mybir.AluOpType
    GR = cfg["gr"]

    nc = bacc.Bacc()
    adjtd = nc.dram_tensor("adjt", (NP, W), mybir.dt.int8, kind="ExternalInput")
    xtd = nc.dram_tensor("xtb", (F, NP), bf16, kind="ExternalInput")
    xwtd = nc.dram_tensor("xwtb", (F, W), bf16, kind="ExternalInput")
    wgd = nc.dram_tensor("wg", (F, 257), bf16, kind="ExternalInput")
    gsd = nc.dram_tensor("gs", (F, 2), bf16, kind="ExternalInput")
    wbd = nc.dram_tensor("wbt", (128, F), f16, kind="ExternalInput")
    cvd = nc.dram_tensor("cv", (128, 1), f32, kind="ExternalInput")
    outd = nc.dram_tensor("out", (W, F), f32, kind="ExternalOutput")

    with tile.TileContext(nc) as tc:
        with (
            tc.tile_pool(name="pp", bufs=1) as pp,
            tc.tile_pool(name="att", bufs=2) as ap_,
            tc.tile_pool(name="fin", bufs=2) as fin,
            tc.tile_pool(name="ps", bufs=8, space="PSUM") as psp,
        ):
            for _rep in range(repeat):
                # ---- phase 0a: parameters and transposed activations ----
                # xt in tile-aligned column chunks on the SP queue so early
                # H' tiles can start while later chunks stream; small params
                # go through the Activation HWDGE queue in parallel.
                wg_sb = [pp.tile([128, 257], bf16, name=f"wg{k}", tag=f"wg{k}") for k in range(2)]
                gs_sb = [pp.tile([128, 2], bf16, name=f"gs{k}", tag=f"gs{k}") for k in range(2)]
                xt = [pp.tile([128, NP], bf16, name=f"xt{k}", tag=f"xt{k}") for k in range(2)]
                xwt = [pp.tile([128, W], bf16, name=f"xwt{k}", tag=f"xwt{k}") for k in range(2)]
                for k in range(2):
                    nc.scalar.dma_start(wg_sb[k][:], wgd[k * 128 : (k + 1) * 128, :])
                    nc.scalar.dma_start(gs_sb[k][:], gsd[k * 128 : (k + 1) * 128, :])
                    nc.scalar.dma_start(xwt[k][:], xwtd[k * 128 : (k + 1) * 128, :])
                XCH = [(0, 2048), (2048, 4096), (4096, 6144), (6144, NP)]
                for c0, c1 in XCH:
                    for k in range(2):
                        nc.sync.dma_start(
                            xt[k][:, c0:c1], xtd[k * 128 : (k + 1) * 128, c0:c1]
                        )
                wb_sb = pp.tile([128, F], f16)
                nc.scalar.dma_start(wb_sb[:], wbd[:])
                cv_sb = pp.tile([128, 1], f32)
                nc.scalar.dma_start(cv_sb[:], cvd[:])

                # ---- phase 0b: s_src row (this core's queries) + consts ----
                ssrc_row = pp.tile([1, W], f32)
                for ch in range(2):
                    ps = psp.tile([2, 512], f32, name="ps_s", tag="ps")
                    for k in range(2):
                        nc.tensor.matmul(
                            ps[:],
                            gs_sb[k][:],
                            xwt[k][:, ch * 512 : (ch + 1) * 512],
                            start=(k == 0),
                            stop=(k == 1),
                        )
                    nc.vector.tensor_scalar_add(
                        ssrc_row[0:1, ch * 512 : (ch + 1) * 512], ps[0:1, :], cv_sb[0:1, 0:1]
                    )

                # ---- phase 0c: broadcast s_src across partitions via PE ----
                ones1 = pp.tile([1, 128], f32)
                nc.vector.memset(ones1[:], 1.0)
                sb1 = pp.tile([128, W], f16)
                for ch in range(2):
                    psb_t = psp.tile([128, 512], f32, name="ps_b", tag="ps")
                    nc.tensor.matmul(
                        psb_t[:], ones1[:], ssrc_row[0:1, ch * 512 : (ch + 1) * 512],
                        start=True, stop=True,
                    )
                    nc.vector.tensor_copy(sb1[:, ch * 512 : (ch + 1) * 512], psb_t[:])

                # ---- phase 0d: H' key tiles ([ones | H | s_dst] per tile) ----
                hp = pp.tile([128, NJT * HPW], f16)
                hp3 = hp[:].rearrange("p (j c) -> p j c", c=HPW)
                nc.vector.memset(hp3[:, :, 0:1], 1.0)
                for jt in range(NJT):
                    ph = psp.tile([128, 257], f32, name="ps_h", tag="ps")
                    for k in range(2):
                        nc.tensor.matmul(
                            ph[:],
                            xt[k][:, jt * 128 : (jt + 1) * 128],
                            wg_sb[k][:],
                            start=(k == 0),
                            stop=(k == 1),
                        )
                    # H' -> cols 1..256, s_dst_raw -> col 257
                    if jt % 2 == 0:
                        nc.vector.tensor_copy(hp[:, jt * HPW + 1 : jt * HPW + 258], ph[:])
                    else:
                        nc.scalar.copy(hp[:, jt * HPW + 1 : jt * HPW + 258], ph[:])

                # dense f32 staging of the s_dst per-partition scalars
                sdst = pp.tile([128, NJT], f32)

                # ---- phase 1: masked attention weights + matmul accumulate ----
                po = [psp.tile([128, 257], f32, name=f"po{ib}", tag="ps") for ib in range(8)]
                groups = [list(range(g, min(g + GR, NJT))) for g in range(0, NJT, GR)]
                for gi, jts in enumerate(groups):
                    na = len(jts)
                    cw = na * W
                    j0 = jts[0]
                    nc.vector.tensor_copy(
                        sdst[:].rearrange("p (j c) -> p j c", c=1)[:, j0 : j0 + na, :],
                        hp3[:, j0 : j0 + na, 257:258],
                    )
                    adj_src = adjtd[j0 * 128 : (j0 + na) * 128, :].rearrange(
                        "(a p) w -> p a w", p=128
                    )
                    u_t = ap_.tile([128, GR * W], f16, name="u_t", tag="u_t", bufs=3)
                    if gi in cfg["act_groups"]:
                        # ACT path: l = prelu(sb1 + s_dst) in one pass per tile
                        l_t = ap_.tile([128, GR * W], f16, name="l_t", tag="l_t", bufs=4)
                        for t, jt in enumerate(jts):
                            nc.scalar.activation(
                                l_t[:, t * W : (t + 1) * W], sb1[:],
                                mybir.ActivationFunctionType.Prelu,
                                bias=sdst[:, jt : jt + 1], scale=1.0, alpha=NEG_SLOPE,
                            )
                    else:
                        # DVE path: e, 0.2e via 4x-mode tensor_scalar, then max
                        e_t = ap_.tile([128, GR * W], f16, name="e_t", tag="e_t", bufs=2)
                        l_t = ap_.tile([128, GR * W], f16, name="l_t", tag="l_t", bufs=4)
                        e2eng = nc.gpsimd if gi in cfg["pool_e2"] else nc.vector
                        for t, jt in enumerate(jts):
                            nc.vector.tensor_scalar_add(
                                e_t[:, t * W : (t + 1) * W], sb1[:], sdst[:, jt : jt + 1]
                            )
                            e2eng.tensor_scalar(
                                l_t[:, t * W : (t + 1) * W], sb1[:],
                                sdst[:, jt : jt + 1], NEG_SLOPE, A.add, A.mult,
                            )
                        if cfg["max_eng"] == "pool":
                            nc.gpsimd.tensor_max(l_t[:, 0:cw], e_t[:, 0:cw], l_t[:, 0:cw])
                        elif cfg["max_eng"] == "split":
                            h = (cw // 2) // W * W or W
                            nc.vector.tensor_max(l_t[:, 0:h], e_t[:, 0:h], l_t[:, 0:h])
                            nc.gpsimd.tensor_max(l_t[:, h:cw], e_t[:, h:cw], l_t[:, h:cw])
                        else:
                            nc.vector.tensor_max(l_t[:, 0:cw], e_t[:, 0:cw], l_t[:, 0:cw])
                    if cfg["mask"] == "cceadd":
                        # additive mask in log space: l += {0,-64}; exp -> 0
                        nc.gpsimd.dma_start(
                            l_t[:].rearrange("p (a w) -> p a w", w=W)[:, 0:na, :],
                            adj_src,
                            accum_op=A.add,
                        )
                        nc.scalar.activation(
                            u_t[:, 0:cw], l_t[:, 0:cw], mybir.ActivationFunctionType.Exp
                        )
                        p_t = u_t
                    else:
                        nc.scalar.activation(
                            u_t[:, 0:cw], l_t[:, 0:cw], mybir.ActivationFunctionType.Exp
                        )
                        adjT = ap_.tile([128, GR * W], f16, name="adjT", tag="adjT", bufs=3)
                        nc.gpsimd.dma_start(
                            adjT[:].rearrange("p (a w) -> p a w", w=W)[:, 0:na, :],
                            adj_src,
                        )
                        p_t = ap_.tile([128, GR * W], f16, name="p_t", tag="p_t", bufs=3)
                        if cfg["mask"] == "pool":
                            nc.gpsimd.tensor_mul(p_t[:, 0:cw], adjT[:, 0:cw], u_t[:, 0:cw])
                        else:
                            nc.vector.tensor_mul(p_t[:, 0:cw], adjT[:, 0:cw], u_t[:, 0:cw])
                    for t, jt in enumerate(jts):
                        for ib in range(8):
                            nc.tensor.matmul(
                                po[ib][:],
                                p_t[:, t * W + ib * MB : t * W + (ib + 1) * MB],
                                hp[:, jt * HPW : jt * HPW + 257],
                                start=(jt == 0),
                                stop=(jt == NJT - 1),
                            )

                # ---- phase 2: normalize + store ----
                for ib in range(8):
                    r = fin.tile([MB, 1], f32, name="rcol", tag="rcol")
                    nc.vector.reciprocal(r[:], po[ib][:, 0:1])
                    ob = fin.tile([MB, F], f32, name="ob", tag="ob")
                    nc.vector.scalar_tensor_tensor(
                        ob[:], po[ib][:, 1:257], r[:], wb_sb[0:MB, :],
                        A.mult, A.add,
                    )
                    nc.sync.dma_start(outd[ib * MB : (ib + 1) * MB, :], ob[:])

    nc.compile()
    return _make_runner(nc, 8)


def _make_runner(nc, n_cores):
    """Compile-once sharded PJRT runner for the 8-core axon path."""
    import time
    import jax
    from jax.sharding import Mesh, PartitionSpec
    from jax.experimental.shard_map import shard_map
    from concourse import mybir
    from concourse.bass2jax import (
        _bass_exec_p,
        install_neuronx_cc_hook,
        partition_id_tensor,
    )

    install_neuronx_cc_hook()
    partition_name = nc.partition_id_tensor.name if nc.partition_id_tensor else None
    in_names, out_names, out_avals, zero_outs = [], [], [], []
    for alloc in nc.m.functions[0].allocations:
        if not isinstance(alloc, mybir.MemoryLocationSet):
            continue
        name = alloc.memorylocations[0].name
        if alloc.kind == "ExternalInput":
            if name != partition_name:
                in_names.append(name)
        elif alloc.kind == "ExternalOutput":
            out_names.append(name)
            shape = tuple(alloc.tensor_shape)
            dtype = mybir.dt.np(alloc.dtype)
            out_avals.append(jax.core.ShapedArray(shape, dtype))
            zero_outs.append(np.zeros(shape, dtype))
    n_params = len(in_names)
    all_in = in_names + out_names + ([partition_name] if partition_name else [])

    def _body(*args):
        operands = list(args)
        if partition_name is not None:
            operands.append(partition_id_tensor())
        return tuple(
            _bass_exec_p.bind(
                *operands,
                out_avals=tuple(out_avals),
                in_names=tuple(all_in),
                out_names=tuple(out_names),
                lowering_input_output_aliases=(),
                sim_require_finite=True,
                sim_require_nnan=True,
                nc=nc,
            )
        )

    devices = jax.devices()[:n_cores]
    mesh = Mesh(np.asarray(devices), ("core",))
    fn = jax.jit(
        shard_map(
            _body,
            mesh=mesh,
            in_specs=(PartitionSpec("core"),) * (n_params + len(out_names)),
            out_specs=(PartitionSpec("core"),) * len(out_names),
            check_rep=False,
        ),
        keep_unused=True,
    )

    def run(in_maps, iters=0):
        per_core = [[np.asarray(m[n]) for n in in_names] for m in in_maps]
        concat_in = [
            np.concatenate([per_core[c][i] for c in range(n_cores)], axis=0)
            for i in range(n_params)
        ]
        concat_zeros = [
            np.zeros((n_cores * z.shape[0], *z.shape[1:]), z.dtype) for z in zero_outs
        ]
        args = [jax.device_put(a) for a in concat_in + concat_zeros]
        out = fn(*args)
        jax.block_until_ready(out)
        times = []
        for _ in range(iters):
            t0 = time.perf_counter()
            out = fn(*args)
            jax.block_until_ready(out)
            times.append(time.perf_counter() - t0)
        results = [
            {
                name: np.asarray(out[i]).reshape(n_cores, *out_avals[i].shape)[c]
                for i, name in enumerate(out_names)
            }
            for c in range(n_cores)
        ]
        return results, (min(times) if times else None)

    return run


def _prep_in_maps(node_embeddings, adj_matrix, W_w, W_b, a_src, a_dst, a_b, mask="cceadd"):
    X = np.asarray(node_embeddings, np.float32)
    adj = np.asarray(adj_matrix, np.int32)
    W_w = np.asarray(W_w, np.float32)
    W_b = np.asarray(W_b, np.float32)
    a_src = np.asarray(a_src, np.float32)
    a_dst = np.asarray(a_dst, np.float32)
    a_b = float(np.asarray(a_b))

    Xb = X.astype(ml_dtypes.bfloat16)
    xtb = np.zeros((F, NP), ml_dtypes.bfloat16)
    xtb[:, :N] = Xb.T
    g_src = (W_w.T @ a_src).astype(np.float32)
    g_dst = (W_w.T @ a_dst).astype(np.float32)
    wg = np.concatenate([W_w.T, g_dst[:, None]], axis=1)
    wg = np.ascontiguousarray(wg).astype(ml_dtypes.bfloat16)
    gs = np.ascontiguousarray(
        np.concatenate([g_src[:, None], g_dst[:, None]], axis=1)
    ).astype(ml_dtypes.bfloat16)
    wbt = np.ascontiguousarray(np.tile(W_b, (128, 1))).astype(np.float16)
    cconst = float(W_b @ a_src + W_b @ a_dst + a_b)
    cv = np.full((128, 1), cconst, np.float32)

    in_maps = []
    for c in range(8):
        r0 = c * W
        rows = min(W, N - r0)
        if mask == "cceadd":
            # {kept: 0, masked: -64}; padding stays masked
            adjt = np.full((NP, W), -64, np.int8)
            adjt[:N, :rows] = (adj[r0 : r0 + rows, :].T.astype(np.int16) - 1).astype(
                np.int8
            ) * 64
        else:
            adjt = np.zeros((NP, W), np.int8)
            adjt[:N, :rows] = adj[r0 : r0 + rows, :].T.astype(np.int8)
        xwtb = np.zeros((F, W), ml_dtypes.bfloat16)
        xwtb[:, :rows] = Xb[r0 : r0 + rows].T
        in_maps.append(
            {
                "adjt": adjt,
                "xtb": xtb,
                "xwtb": xwtb,
                "wg": wg,
                "gs": gs,
                "wbt": wbt,
                "cv": cv,
            }
        )
    return in_maps


def kernel(node_embeddings, adj_matrix, W_w, W_b, a_src, a_dst, a_b):
    global _RUNNER, _last_in_maps
    if _RUNNER is None:
        _RUNNER = _build()

    in_maps = _prep_in_maps(
        node_embeddings, adj_matrix, W_w, W_b, a_src, a_dst, a_b, mask=_cfg()["mask"]
    )
    _last_in_maps = in_maps
    results, _ = _RUNNER(in_maps, iters=0)
    out = np.empty((N, F), np.float32)
    for c in range(8):
        r0 = c * W
        rows = min(W, N - r0)
        out[r0 : r0 + rows] = results[c]["out"][:rows]
    return out
